# revision 30
# baseline (speedup 1.0000x reference)
"""DeepFM forward on Trainium2, 8 NeuronCores, data-parallel over batch.

Reference computes (B=512, n=512, K=4, H=128, n_pairs=130816):
    S  = fm_w @ fm_w.T
    fm = x[:, i1] * x[:, i2] * S[i1, i2]        # [B, n_pairs]
    h2 = relu(relu(x@w1+b1)@w2+b2)
    out = sigmoid(concat([fm, h2]) @ wo + bo)

The fm @ wo[:n_pairs] contraction is the bilinear form
    t1[b] = x[b]^T Wp' x[b]  with  Wp'[i,j] = S[i,j] * Wp[i,j]
where Wp is wo[:n_pairs] scattered into the strictly-upper triangle of a
[n, n] matrix (a pure re-layout of wo done on host; indices are static).
Since S = fm_w @ fm_w.T has rank 4, this factors as
    t1[b] = sum_t z_t[b]^T Wp z_t[b],  z_t = x * fm_w[:, t]
so the device never materializes S: Wp is used directly as the matmul
operand and the rank-4 scaling is cheap broadcast DVE work. Wp is
strictly upper triangular, so only the 10 upper-triangular 128x128
blocks are shipped and multiplied (the 6 lower blocks are zero).

All inputs are repacked on host into per-partition-contiguous [128, X]
SBUF images so each dma_start moves 128 fat contiguous runs (the SDMA
per-packet cost dominates latency otherwise). The critical small
tensors (x shard, fm_w, biases) ride one early DMA on the sync queue;
the f32 section lives in the bf16 image via bitcast.

Per-core program (batch shard = 64 columns, feature-on-partition layout,
bf16 operands / fp32 accumulation; t stacked along the free dim):
    Z_k[:, t, :]  = xT_k * fm_w[k-chunk, t]       (DVE broadcast mul)
    VT_j = sum_{k<=j} Wp[k128, j128]^T @ Z_k      (PE, j-major blocks)
    Q_j  = VT_j * Z_j                             (DVE, bf16 out)
    t    = sum_{j,t} Q_j[:,t,:]^T @ ones + h2^T @ wo_h  (PE psum accum) [64,1]
    h1   = max(w1^T @ xT + b1, 0)                 (PE+DVE)
    h2   = max(w2^T @ h1 + b2, 0)                 (PE+DVE)
    out  = sigmoid(t + bo)                        (ACT, table pre-warmed)

The PE is HAM-warmed with dummy matmuls on memset tiles during the DMA
wait so the back half of the kernel runs at the fast clock.
"""

import os
import sys

import numpy as np

for _p in ("/opt/trn_rl_repo", "/root/.axon_site/_ro/trn_rl_repo"):
    if os.path.isdir(_p) and _p not in sys.path:
        sys.path.insert(0, _p)

import ml_dtypes

import concourse.bass as bass
import concourse.tile as tile
from concourse import bacc, mybir
from concourse.bass import ts
from concourse.bass_utils import run_bass_kernel_spmd

F32 = mybir.dt.float32
BF16 = mybir.dt.bfloat16
AF = mybir.ActivationFunctionType
ALU = mybir.AluOpType

N = 512          # n_feat
KFM = 4          # fm embedding dim
H = 128          # mlp hidden
NP = N * (N - 1) // 2
B = 512
N_CORES = 8
BC = B // N_CORES  # 64 batch rows per core
NCH = N // 128     # 4 feature chunks
N_WARM = int(os.environ.get("DFM_N_WARM", "16"))  # PE warm-up dummy matmuls

# Upper-triangular 128x128 blocks of Wp in j-major order.
UBLOCKS = [(k, j) for j in range(NCH) for k in range(j + 1)]
UB_OFF = {kj: i * 128 for i, kj in enumerate(UBLOCKS)}  # column offset in image
WP_COLS = len(UBLOCKS) * 128  # 1280
WP_SPLIT = UB_OFF[(0, 2)]     # j0+j1 blocks first, then j2+j3's

# f32 pack layout (viewed at [128, 20] f32): [fmw (4*4) | b1 | b2 | woh | bo]
FM_OFF = 0
PK_OFF = FM_OFF + NCH * KFM
F32_COLS = PK_OFF + 4      # 20
# crit image (bf16): [xt (4*64) | f32 pack as raw bf16 pairs (40)]
XT_OFF = 0
FP_OFF = NCH * BC          # 256
CRIT_COLS = FP_OFF + F32_COLS * 2  # 296

_IU1, _IU2 = np.triu_indices(N, k=1)

_program_cache = None


def _chunk_pack(a, cols):
    """[512, cols] row-major -> [128, 4*cols] with chunk c at column block c."""
    return np.ascontiguousarray(
        a.reshape(NCH, 128, cols).transpose(1, 0, 2).reshape(128, NCH * cols)
    )


def _build_program():
    global _program_cache
    if _program_cache is not None:
        return _program_cache

    nc = bacc.Bacc(
        "TRN2", target_bir_lowering=False, debug=False, num_devices=N_CORES
    )
    crit_d = nc.declare_dram_parameter("crit", [128, CRIT_COLS], BF16, isOutput=False)
    wp_d = nc.declare_dram_parameter("wp", [128, WP_COLS], BF16, isOutput=False)
    w12_d = nc.declare_dram_parameter(
        "w12", [128, NCH * H + H], BF16, isOutput=False
    )
    out_d = nc.declare_dram_parameter("out", [1, BC], F32, isOutput=True)

    with tile.TileContext(nc) as tc:
        with (
            tc.tile_pool(name="const", bufs=1) as cpool,
            tc.tile_pool(name="work", bufs=1) as wpool,
            tc.tile_pool(name="ps_v", bufs=1, space=bass.MemorySpace.PSUM) as vpool,
            tc.tile_pool(name="ps_h", bufs=1, space=bass.MemorySpace.PSUM) as hpool,
            tc.tile_pool(name="ps_t", bufs=1, space=bass.MemorySpace.PSUM) as tpool,
        ):
            # ---- loads. sync queue: crit first, then the Wp halves ----
            crit_sb = cpool.tile([128, CRIT_COLS], BF16)
            nc.sync.dma_start(crit_sb[:], crit_d[:, :])
            wp_sb = cpool.tile([128, WP_COLS], BF16)
            s2, s3 = UB_OFF[(0, 2)], UB_OFF[(0, 3)]
            nc.sync.dma_start(wp_sb[:, :s2], wp_d[:, :s2])
            nc.sync.dma_start(wp_sb[:, s2:s3], wp_d[:, s2:s3])
            nc.sync.dma_start(wp_sb[:, s3:], wp_d[:, s3:])
            w12_sb = cpool.tile([128, NCH * H + H], BF16)
            nc.scalar.dma_start(w12_sb[:], w12_d[:, :])

            f32v = crit_sb[:, FP_OFF:].bitcast(F32)  # [128, 20] f32 view

            def xt(k):
                return crit_sb[:, XT_OFF + k * BC : XT_OFF + (k + 1) * BC]

            def w1c(k):
                return w12_sb[:, k * H : (k + 1) * H]

            w2_ap = w12_sb[:, NCH * H : NCH * H + H]
            b1_ap = f32v[:, PK_OFF : PK_OFF + 1]
            b2_ap = f32v[:, PK_OFF + 1 : PK_OFF + 2]
            woh_ap = f32v[:, PK_OFF + 2 : PK_OFF + 3]
            bo_ap = f32v[0:1, PK_OFF + 3 : PK_OFF + 4]

            # ---- constants (Vector memsets — fast, idle early) ----
            dum_lhs = cpool.tile([128, 128], BF16)
            nc.vector.memset(dum_lhs[:], 0.0)
            dum_rhs = cpool.tile([128, KFM * BC], BF16)
            nc.vector.memset(dum_rhs[:], 0.0)
            ones_sb = cpool.tile([128, 1], BF16)
            nc.vector.memset(ones_sb[:], 1.0)
            warm_in = cpool.tile([1, 1], F32)
            nc.vector.memset(warm_in[:], 0.0)
            warm_out = cpool.tile([1, 1], F32)
            nc.scalar.activation(warm_out[:], warm_in[:], AF.Sigmoid, bias=0.0)

            # ---- PE HAM warm-up into the (late-used) MLP/t psum banks ----
            dum_tags = ["h1_ps", "h2_ps", "t_ps"]
            for d in range(N_WARM):
                dum_ps = hpool.tile(
                    [128, KFM * BC], F32, name=f"dum{d}",
                    tag=dum_tags[d % 2],
                )
                nc.tensor.matmul(
                    dum_ps[:], dum_lhs[:], dum_rhs[:], start=True, stop=True
                )

            # ---- Z_k[:, t, :] = xT_k scaled by fm_w column t (rank-4) ----
            z_tiles = []
            for k in range(NCH):
                z_sb = wpool.tile([128, KFM, BC], BF16, name=f"z{k}", tag=f"z{k}")
                nc.vector.tensor_mul(
                    z_sb[:],
                    xt(k)[:, None, :].broadcast_to([128, KFM, BC]),
                    f32v[:, FM_OFF + k * KFM : FM_OFF + (k + 1) * KFM][
                        :, :, None
                    ].broadcast_to([128, KFM, BC]),
                )
                z_tiles.append(z_sb)

            # ---- VT_j = sum_{k<=j} Wp[k,j]^T @ Z_k (upper blocks only) ----
            vt_tiles = [
                vpool.tile([128, KFM, BC], F32, name=f"vt{j}", tag=f"v{j}")
                for j in range(NCH)
            ]
            for j in range(NCH):
                for k in range(j + 1):
                    off = UB_OFF[(k, j)]
                    nc.tensor.matmul(
                        vt_tiles[j][:], wp_sb[:, off : off + 128], z_tiles[k][:],
                        start=(k == 0), stop=(k == j),
                    )

            # ---- MLP ----
            h1_ps = hpool.tile([H, BC], F32)
            for k in range(NCH):
                nc.tensor.matmul(
                    h1_ps[:], w1c(k), xt(k),
                    start=(k == 0), stop=(k == NCH - 1),
                )
            h1_sb = wpool.tile([H, BC], BF16)
            nc.vector.tensor_scalar(
                h1_sb[:], h1_ps[:], b1_ap, 0.0, op0=ALU.add, op1=ALU.max
            )
            h2_ps = hpool.tile([H, BC], F32)
            nc.tensor.matmul(h2_ps[:], w2_ap, h1_sb[:], start=True, stop=True)
            h2_sb = wpool.tile([H, BC], F32)
            nc.vector.tensor_scalar(
                h2_sb[:], h2_ps[:], b2_ap, 0.0, op0=ALU.add, op1=ALU.max
            )

            # ---- Q_j = VT_j * Z_j; fold partitions and t into t_ps [1, 64] ----
            t_ps = tpool.tile([1, BC], F32, tag="t_ps")
            for j in range(NCH):
                q_sb = wpool.tile([128, KFM, BC], BF16, name=f"q{j}", tag=f"q{j}")
                nc.vector.tensor_mul(q_sb[:], vt_tiles[j][:], z_tiles[j][:])
                for t in range(KFM):
                    nc.tensor.matmul(
                        t_ps[:], ones_sb[:], q_sb[:, t, :],
                        start=(j == 0 and t == 0), stop=False,
                    )
            nc.tensor.matmul(t_ps[:], woh_ap, h2_sb[:], start=False, stop=True)

            out_sb = wpool.tile([1, BC], F32)
            nc.scalar.activation(out_sb[:], t_ps[:], AF.Sigmoid, bias=bo_ap)
            nc.scalar.dma_start(out_d[:, :], out_sb[:])

    nc.compile()
    _program_cache = nc
    return nc


def _prep_inputs(x, fm_w, w1, b1, w2, b2, wo, bo):
    x = np.asarray(x, dtype=np.float32)
    fm_w = np.asarray(fm_w, dtype=np.float32)
    w1 = np.asarray(w1, dtype=np.float32)
    w2 = np.asarray(w2, dtype=np.float32)
    wo = np.asarray(wo, dtype=np.float32).reshape(NP + H)
    b1 = np.asarray(b1, dtype=np.float32).reshape(H)
    b2 = np.asarray(b2, dtype=np.float32).reshape(H)
    bo = np.asarray(bo, dtype=np.float32).reshape(1)

    bf = ml_dtypes.bfloat16

    # Scatter pair weights into the strictly-upper triangle (static index
    # relayout, same (j1, j2>j1) row-major order as the reference), then
    # pack only the upper-triangular 128x128 blocks, j-major.
    wp = np.zeros((N, N), dtype=np.float32)
    wp[_IU1, _IU2] = wo[:NP]
    wp_bf = wp.astype(bf)
    wp_img = np.empty((128, WP_COLS), dtype=bf)
    for (k, j), off in UB_OFF.items():
        wp_img[:, off : off + 128] = wp_bf[
            128 * k : 128 * (k + 1), 128 * j : 128 * (j + 1)
        ]
    wp_img = np.ascontiguousarray(wp_img)

    w12_img = np.empty((128, NCH * H + H), dtype=bf)
    w12_img[:, : NCH * H] = _chunk_pack(w1.astype(bf), H)
    w12_img[:, NCH * H :] = w2.astype(bf)
    w12_img = np.ascontiguousarray(w12_img)

    f32_img = np.zeros((128, F32_COLS), dtype=np.float32)
    f32_img[:, FM_OFF : FM_OFF + NCH * KFM] = _chunk_pack(fm_w, KFM)
    f32_img[:, PK_OFF] = b1
    f32_img[:, PK_OFF + 1] = b2
    f32_img[:, PK_OFF + 2] = wo[NP:]
    f32_img[:, PK_OFF + 3] = bo[0]   # replicated: per-partition sigmoid bias

    xT = x.T.astype(bf)                                         # [512, 512]

    in_maps = []
    for c in range(N_CORES):
        crit = np.empty((128, CRIT_COLS), dtype=bf)
        crit[:, XT_OFF:FP_OFF] = _chunk_pack(
            np.ascontiguousarray(xT[:, c * BC : (c + 1) * BC]), BC
        )
        crit[:, FP_OFF:] = f32_img.view(bf)   # raw f32 bytes as bf16 pairs
        in_maps.append(
            {
                "crit": np.ascontiguousarray(crit),
                "wp": wp_img,
                "w12": w12_img,
            }
        )
    return in_maps


def run(inputs, **spmd_kwargs):
    """Build, run on 8 cores, return (output [512,1] f32, BassKernelResults)."""
    nc = _build_program()
    in_maps = _prep_inputs(**inputs)
    res = run_bass_kernel_spmd(nc, in_maps, list(range(N_CORES)), **spmd_kwargs)
    out = np.concatenate(
        [res.results[c]["out"].reshape(BC) for c in range(N_CORES)]
    ).reshape(B, 1).astype(np.float32)
    return out, res


def kernel(**inputs) -> np.ndarray:
    out, _ = run(inputs)
    return out
